# revision 11
# baseline (speedup 1.0000x reference)
"""Trainium2 Bass kernel for CrossAttention (B=8, L=M=1024, D=768, H=8).

Sharding: data-parallel over batch — core b computes batch element b fully.
No collectives.

Per-core pipeline (all-transposed "T-layout"):
  1. LayerNorm x, context in natural layout [l_part, d_free] (bn_stats).
  2. PE-transpose normalized z -> zT [d_part, l_free] (f32r, identity).
  3. Projections (f32r matmuls, N=512 moving):
       qT_h [96, L]  = (g1*Wq*SCALE)_h^T zT_x    (head-wise, K=128 chunks)
       kT_h [96, M]  = (g2*Wk)_h^T zT_c
       v natural [m_tile, 8*(96+1)] bf16, with a ones column per head block.
  4. Attention per (head, l_piece): attnT psum [m_chunk, 512] = kT_h^T qT_h;
     ACT exp -> bf16 SBUF; wv matmul lhsT=v_aug (includes ones col) accumulates
     [97, 512] psum: rows 0-95 raw out^T, row 96 = softmax denominator.
  5. recip + K=1 matmul broadcast -> normalize -> normT_h bf16 [96, L].
  6. Final proj per l_tile: psum [128, 768] accumulates 8 heads (bf16 MMs),
     + residual x -> out.
attn_map: softmax rows sum to 1 -> mean over heads of row-sums == 1.0
exactly; computed on host as ones (matches reference to ~1e-6).
Biases/g folding: g1,g2 folded into weights on host; all biases in this
problem are zero (asserted).
"""
import sys

sys.path.insert(0, "/opt/trn_rl_repo")
import numpy as np

DIM = 768
HEADS = 8
HD = 96  # head dim
SCALE = HD ** -0.5
EPS = 1e-5
P = 128  # partitions
DC = DIM // P  # 6 d-chunks
PIECE = 512  # moving free dim per matmul

_compiled_cache = {}


def build_nc(L=1024, M=1024, num_devices=8):
    import concourse.bass as bass
    import concourse.tile as tile
    from concourse import bacc, mybir
    from contextlib import ExitStack

    f32 = mybir.dt.float32
    f32r = mybir.dt.float32r
    bf16 = mybir.dt.bfloat16
    AF = mybir.ActivationFunctionType
    ALU = mybir.AluOpType

    NT = L // P      # l tiles
    MC = M // P      # m chunks/tiles
    LP = L // PIECE  # l pieces
    assert L % PIECE == 0 and M % P == 0 and L % P == 0

    def r(ap):
        return ap

    nc = bacc.Bacc("TRN2", target_bir_lowering=False, debug=False,
                   num_devices=num_devices)

    x_d = nc.dram_tensor("x", [L, DIM], f32, kind="ExternalInput").ap()
    c_d = nc.dram_tensor("ctx", [M, DIM], f32, kind="ExternalInput").ap()
    wq_d = nc.dram_tensor("wq", [DIM, DIM], f32r, kind="ExternalInput").ap()
    wk_d = nc.dram_tensor("wk", [DIM, DIM], f32r, kind="ExternalInput").ap()
    wv_d = nc.dram_tensor("wv", [DIM, DIM], f32r, kind="ExternalInput").ap()
    wp_d = nc.dram_tensor("wp", [HEADS, HD, DIM], bf16, kind="ExternalInput").ap()
    id_d = nc.dram_tensor("ident", [P, P], f32r, kind="ExternalInput").ap()
    on_d = nc.dram_tensor("ones96", [1, HD], f32r, kind="ExternalInput").ap()
    out_d = nc.dram_tensor("out", [L, DIM], f32, kind="ExternalOutput").ap()

    x_t = x_d.rearrange("(t p) d -> t p d", p=P)
    c_t = c_d.rearrange("(t p) d -> t p d", p=P)
    wq_t = wq_d.rearrange("(c p) f -> c p f", p=P)
    wk_t = wk_d.rearrange("(c p) f -> c p f", p=P)
    wv_t = wv_d.rearrange("(c p) f -> c p f", p=P)
    out_t = out_d.rearrange("(t p) d -> t p d", p=P)

    with tile.TileContext(nc) as tc, \
            nc.allow_low_precision(reason="f32r activations for full-rate PE"), \
            ExitStack() as ctx:
        persist = ctx.enter_context(tc.tile_pool(name="persist", bufs=1))
        work = ctx.enter_context(tc.tile_pool(name="work", bufs=3))
        stats = ctx.enter_context(tc.tile_pool(name="stats", bufs=4))

        # ---- constants ----
        ident = persist.tile([P, P], f32r, tag="ident")
        nc.sync.dma_start(ident[:], id_d[:])
        ones96 = persist.tile([1, HD], f32r, tag="ones96")
        nc.sync.dma_start(ones96[:], on_d[:])

        # ---- residual x tiles (persist) ----
        xres = []
        for t in range(NT):
            xt = persist.tile([P, DIM], f32, tag=f"xres{t}")
            nc.sync.dma_start(xt[:], x_t[t])
            xres.append(xt)

        def layernorm(src_ap, dst_tile):
            """src [128, DIM] (SBUF) -> dst z tile [128, DIM] f32"""
            bn6 = stats.tile([P, 2, 6], f32, tag="bn6")
            nc.vector.bn_stats(bn6[:, 0, :], src_ap[:, 0:384])
            nc.vector.bn_stats(bn6[:, 1, :], src_ap[:, 384:768])
            mv = stats.tile([P, 2], f32, tag="mv")
            nc.vector.bn_aggr(mv[:], bn6[:])
            veps = stats.tile([P, 1], f32, tag="veps")
            nc.vector.tensor_scalar_add(veps[:], mv[:, 1:2], EPS)
            std = stats.tile([P, 1], f32, tag="std")
            nc.scalar.activation(std[:], veps[:], AF.Sqrt)
            rstd = stats.tile([P, 1], f32, tag="rstd")
            nc.vector.reciprocal(rstd[:], std[:])
            nc.vector.tensor_scalar(dst_tile[:], src_ap[:], mv[:, 0:1],
                                    rstd[:], ALU.subtract, ALU.mult)

        def transpose_to(zT_tiles, z_tile, t, ps_pool):
            """z [128l, DIM] tile t -> zT_tiles[dc][:, t*128:(t+1)*128]"""
            for dc in range(DC):
                pst = ps_pool.tile([P, P], f32r, tag="tp")
                nc.tensor.transpose(r(pst[:]),
                                    r(z_tile[:, dc * P:(dc + 1) * P]),
                                    r(ident[:]))
                nc.vector.tensor_copy(
                    zT_tiles[dc][:, t * P:(t + 1) * P], pst[:])

        # ================= x side: LN -> zxT -> qT =================
        qT = [persist.tile([HD, L], f32r, tag=f"qT{h}", name=f"qT{h}") for h in range(HEADS)]
        with ExitStack() as xctx:
            zxT_pool = xctx.enter_context(tc.tile_pool(name="zxT", bufs=1))

            wq_pool = xctx.enter_context(tc.tile_pool(name="wq", bufs=1))
            zxT = [zxT_pool.tile([P, L], f32r, tag=f"zxT{dc}", name=f"zxT{dc}")
                   for dc in range(DC)]
            wq_sb = []
            for dc in range(DC):
                wt = wq_pool.tile([P, DIM], f32r, tag=f"wq{dc}")
                nc.sync.dma_start(wt[:], wq_t[dc])
                wq_sb.append(wt)
            with tc.tile_pool(name="tp_ps", bufs=4, space="PSUM") as tp_ps:
                for t in range(NT):
                    z = work.tile([P, DIM], f32r, tag="z")
                    layernorm(xres[t][:], z)
                    transpose_to(zxT, z, t, tp_ps)
            # q projection (head-wise): psum [96, 512] acc over 6 d-chunks
            qps_pool = xctx.enter_context(
                tc.tile_pool(name="q_ps", bufs=3, space="PSUM"))
            for h in range(HEADS):
                for pc in range(LP):
                    qps = qps_pool.tile([HD, PIECE], f32, tag="qps")
                    for dc in range(DC):
                        nc.tensor.matmul(
                            qps[:],
                            r(wq_sb[dc][:, h * HD:(h + 1) * HD]),
                            r(zxT[dc][:, pc * PIECE:(pc + 1) * PIECE]),
                            start=(dc == 0), stop=(dc == DC - 1))
                    nc.vector.tensor_copy(
                        qT[h][:, pc * PIECE:(pc + 1) * PIECE], qps[:])

        # ================= c side: LN -> zcT -> kT, v =================
        kT = [persist.tile([HD, M], f32r, tag=f"kT{h}", name=f"kT{h}") for h in range(HEADS)]
        VW = HEADS * (HD + 1)  # 776
        v_aug = [persist.tile([P, VW], bf16, tag=f"vaug{m}", name=f"vaug{m}")
                 for m in range(MC)]
        with ExitStack() as cctx:
            zcT_pool = cctx.enter_context(tc.tile_pool(name="zcT", bufs=1))

            wkv_pool = cctx.enter_context(tc.tile_pool(name="wkv", bufs=1))
            zcT = [zcT_pool.tile([P, M], f32r, tag=f"zcT{dc}", name=f"zcT{dc}")
                   for dc in range(DC)]
            wk_sb, wv_sb = [], []
            for dc in range(DC):
                wt = wkv_pool.tile([P, DIM], f32r, tag=f"wk{dc}")
                nc.sync.dma_start(wt[:], wk_t[dc])
                wk_sb.append(wt)
                wt = wkv_pool.tile([P, DIM], f32r, tag=f"wv{dc}")
                nc.sync.dma_start(wt[:], wv_t[dc])
                wv_sb.append(wt)
            with tc.tile_pool(name="tp_ps2", bufs=4, space="PSUM") as tp_ps:
                for t in range(MC):
                    cin = work.tile([P, DIM], f32, tag="io")
                    nc.sync.dma_start(cin[:], c_t[t])
                    z = work.tile([P, DIM], f32r, tag="z")
                    layernorm(cin[:], z)
                    transpose_to(zcT, z, t, tp_ps)
            kps_pool = cctx.enter_context(
                tc.tile_pool(name="k_ps", bufs=3, space="PSUM"))
            for h in range(HEADS):
                for pc in range(M // PIECE):
                    kps = kps_pool.tile([HD, PIECE], f32, tag="kps")
                    for dc in range(DC):
                        nc.tensor.matmul(
                            kps[:],
                            r(wk_sb[dc][:, h * HD:(h + 1) * HD]),
                            r(zcT[dc][:, pc * PIECE:(pc + 1) * PIECE]),
                            start=(dc == 0), stop=(dc == DC - 1))
                    nc.vector.tensor_copy(
                        kT[h][:, pc * PIECE:(pc + 1) * PIECE], kps[:])
            # v natural: psum [128, 768] = z_c[m_tile] @ Wv
            vps_pool = cctx.enter_context(
                tc.tile_pool(name="v_ps", bufs=2, space="PSUM"))
            for m in range(MC):
                vps = vps_pool.tile([P, DIM], f32, tag="vps")
                for dc in range(DC):
                    nc.tensor.matmul(
                        vps[:, 0:512],
                        r(zcT[dc][:, m * P:(m + 1) * P]),
                        r(wv_sb[dc][:, 0:512]),
                        start=(dc == 0), stop=(dc == DC - 1))
                    nc.tensor.matmul(
                        vps[:, 512:768],
                        r(zcT[dc][:, m * P:(m + 1) * P]),
                        r(wv_sb[dc][:, 512:768]),
                        start=(dc == 0), stop=(dc == DC - 1))
                # scatter heads into v_aug blocks of 97, ones col last
                va3 = v_aug[m][:].rearrange("p (h c) -> p h c", c=HD + 1)
                nc.vector.tensor_copy(
                    va3[:, :, 0:HD],
                    vps[:].rearrange("p (h c) -> p h c", c=HD))
                nc.vector.memset(va3[:, :, HD:HD + 1], 1.0)

        # ================= attention =================
        normT = [persist.tile([HD, L], bf16, tag=f"nT{h}", name=f"nT{h}")
                 for h in range(HEADS)]
        with ExitStack() as actx:
            a_ps = actx.enter_context(
                tc.tile_pool(name="a_ps", bufs=3, space="PSUM"))
            wv_ps = actx.enter_context(
                tc.tile_pool(name="wv_ps", bufs=2, space="PSUM"))
            bc_ps = actx.enter_context(
                tc.tile_pool(name="bc_ps", bufs=2, space="PSUM"))
            exps_pool = actx.enter_context(tc.tile_pool(name="exps", bufs=4))
            rc_pool = actx.enter_context(tc.tile_pool(name="rc", bufs=3))
            for h in range(HEADS):
                for pc in range(LP):
                    ops = wv_ps.tile([HD + 1, PIECE], f32, tag="ops")
                    for mc in range(MC):
                        aps = a_ps.tile([P, PIECE], f32, tag="aps")
                        nc.tensor.matmul(
                            aps[:],
                            r(kT[h][:, mc * P:(mc + 1) * P]),
                            r(qT[h][:, pc * PIECE:(pc + 1) * PIECE]),
                            start=True, stop=True)
                        et = exps_pool.tile([P, PIECE], bf16, tag="exps")
                        nc.scalar.activation(et[:], aps[:], AF.Exp)
                        nc.tensor.matmul(
                            ops[:],
                            v_aug[mc][:, h * (HD + 1):(h + 1) * (HD + 1)],
                            et[:],
                            start=(mc == 0), stop=(mc == MC - 1))
                    # softmax denominator -> reciprocal -> broadcast
                    rc = rc_pool.tile([HD + 1, PIECE], f32r, tag="rc")
                    nc.vector.reciprocal(rc[HD:HD + 1, :], ops[HD:HD + 1, :])
                    nc.vector.tensor_copy(rc[0:1, :], rc[HD:HD + 1, :])
                    bc = bc_ps.tile([HD, PIECE], f32, tag="bc")
                    nc.tensor.matmul(bc[:], r(ones96[:]), r(rc[0:1, :]),
                                     start=True, stop=True)
                    bcs = rc_pool.tile([HD, PIECE], f32, tag="bcs")
                    nc.scalar.copy(bcs[:], bc[:])
                    nc.vector.tensor_tensor(
                        normT[h][:, pc * PIECE:(pc + 1) * PIECE],
                        ops[0:HD, :], bcs[:], ALU.mult)

        # ================= output projection + residual =================
        with ExitStack() as octx:
            wp_pool = octx.enter_context(tc.tile_pool(name="wp", bufs=1))
            wp_sb = []
            for h in range(HEADS):
                wt = wp_pool.tile([HD, DIM], bf16, tag=f"wp{h}")
                nc.sync.dma_start(wt[:], wp_d[h])
                wp_sb.append(wt)
            o_ps = octx.enter_context(
                tc.tile_pool(name="o_ps", bufs=2, space="PSUM"))
            for t in range(NT):
                ops_ = o_ps.tile([P, DIM], f32, tag="ops_")
                for h in range(HEADS):
                    nc.tensor.matmul(
                        ops_[:, 0:512],
                        normT[h][:, t * P:(t + 1) * P],
                        wp_sb[h][:, 0:512],
                        start=(h == 0), stop=(h == HEADS - 1))
                    nc.tensor.matmul(
                        ops_[:, 512:768],
                        normT[h][:, t * P:(t + 1) * P],
                        wp_sb[h][:, 512:768],
                        start=(h == 0), stop=(h == HEADS - 1))
                osb = work.tile([P, DIM], f32, tag="io")
                nc.vector.tensor_tensor(osb[:], ops_[:], xres[t][:], ALU.add)
                nc.sync.dma_start(out_t[t], osb[:])

    nc.compile()
    return nc


def prep_inputs(x, context, Wq, bq, Wkv, bkv, Wp, bp, g1, b1, g2, b2):
    """Host-side weight folding. Returns per-core in_maps."""
    import ml_dtypes

    for b_, name in ((bq, "bq"), (bkv, "bkv"), (bp, "bp"), (b1, "b1"),
                     (b2, "b2")):
        assert np.abs(np.asarray(b_)).max() == 0.0, f"nonzero {name} unsupported"
    g1 = np.asarray(g1, np.float32)
    g2 = np.asarray(g2, np.float32)
    Wq_eff = (np.asarray(Wq, np.float32) * g1[:, None] * SCALE).astype(np.float32)
    Wk_eff = (np.asarray(Wkv[:, :DIM], np.float32) * g2[:, None]).astype(np.float32)
    Wv_eff = (np.asarray(Wkv[:, DIM:], np.float32) * g2[:, None]).astype(np.float32)
    Wp_h = np.ascontiguousarray(
        np.asarray(Wp, np.float32).reshape(HEADS, HD, DIM)
    ).astype(ml_dtypes.bfloat16)
    ident = np.eye(P, dtype=np.float32)
    ones96 = np.ones((1, HD), np.float32)
    B = x.shape[0]
    in_maps = []
    for b in range(B):
        in_maps.append({
            "x": np.ascontiguousarray(np.asarray(x[b], np.float32)),
            "ctx": np.ascontiguousarray(np.asarray(context[b], np.float32)),
            "wq": Wq_eff, "wk": Wk_eff, "wv": Wv_eff, "wp": Wp_h,
            "ident": ident, "ones96": ones96,
        })
    return in_maps


def kernel(x, context, Wq, bq, Wkv, bkv, Wp, bp, g1, b1, g2, b2):
    from concourse import bass_utils

    B, L, D = x.shape
    M = context.shape[1]
    key = (L, M)
    if key not in _compiled_cache:
        _compiled_cache[key] = build_nc(L=L, M=M, num_devices=B)
    nc = _compiled_cache[key]
    in_maps = prep_inputs(x, context, Wq, bq, Wkv, bkv, Wp, bp, g1, b1, g2, b2)
    res = bass_utils.run_bass_kernel_spmd(nc, in_maps, core_ids=list(range(B)))
    out = np.stack([res.results[b]["out"] for b in range(B)], axis=0)
    hp = int(L ** 0.5)
    attn_map = np.ones((B, hp, hp), np.float32)
    return out, attn_map
